# revision 11
# baseline (speedup 1.0000x reference)
"""BiMatchLoss kernel for Trainium2 (8 NeuronCores, SPMD data-parallel over batch).

Math (validated vs reference):
  BCE(p,t) = -log1mp(p) - t*(logp(p) - log1mp(p))
  Summed over a bijective matching perm, the -log1mp part is perm-independent.
  Per batch b the device computes (one pass over the data):
    cost[t,o]  = -sum_{s,ci} tgt[s,t,ci] * out[s,o,ci]            (argmin input)
    G[t,o]     =  sum_{s,ci} tgt[s,t,ci] * mD[s,o,ci]
    Amask      =  sum_{s,o,ci} m[s] * (-log1mp[s,o,ci])
  where mD = m*(logp - log1mp). Host pre-masks the Ln inputs so the device
  computes m*logp = Ln(m*p + 1-m) and m*log1mp = Ln(m*(1-p) + 1-m) directly
  (the (1-p) form keeps fp8 inputs accurate where p -> 1).
  final = sum_b 0.5*(Amask_b - sum_t G[t, perm_b[t]]) / sum(m)

Device per batch: 2 Ln activations over [128,1536] fp8 inputs (ACT; the
log1mp one accumulates Amask row-sums), a subtract writing fp8 mD into the
comb rhs slots, 8 fp8 DoubleRow matmuls (K=256 = two s-tiles per matmul,
PSUM-accumulated over 4 double-tiles; the t4,t5 weight chunk is zero-padded to
M=128 so every PSUM row is written), then ONE PSUM->SBUF f16 copy. The
block-diagonal extraction needs no on-device reduce at all: the wanted
entry per partition is the single element PS[p, g*32 + p%32], so the host
just fancy-indexes the copied [128,768] panes and sums 32 rows. All input
DMAs ride the sync queue as contiguous per-partition lines in exact
consumption order so the ACT chain never stalls. Batch 3's log1mp/sub/
matmuls are split 6+2 tiles and its PSUM copy runs on the then-idle ACT
engine to shorten the serial tail. Host does the 720-permutation argmin and
final scalar assembly.
"""

import os
from itertools import permutations

import numpy as np
import ml_dtypes

import concourse.bacc as bacc
import concourse.mybir as mybir
from concourse.tile import TileContext
from concourse.bass_utils import run_bass_kernel_spmd

B, S, E, C = 32, 1024, 6, 16
F = E * C * 2          # 192 flattened (e, c, i)
CI = C * 2             # 32
NCORE = 8
NB = B // NCORE        # 4 batches per core
NT = S // 128          # 8 s-tiles per batch
ND = NT // 2           # 4 double-tiles (K=256) per batch

f32 = mybir.dt.float32
f16 = mybir.dt.float16
bf16 = mybir.dt.bfloat16
fp8 = mybir.dt.float8e4
AF = mybir.ActivationFunctionType
ALU = mybir.AluOpType
DR = mybir.MatmulPerfMode.DoubleRow

_PROG = None           # cached compiled Bass program
LAST = None            # last BassKernelResults (for test.py timing)


def _build_program():
    nc = bacc.Bacc("TRN2", target_bir_lowering=False, debug=False,
                   num_devices=1)

    lnin_d = nc.dram_tensor("lnin", [128, NB * 1920], fp8,
                            kind="ExternalInput").ap()
    xt_d = nc.dram_tensor("xt", [128, NB * 2048], fp8,
                          kind="ExternalInput").ap()
    xoc_d = nc.dram_tensor("xoc", [128, NB * 3072], fp8,
                           kind="ExternalInput").ap()
    cop_d = nc.dram_tensor("cop", [128, NB * 768], f16,
                           kind="ExternalOutput").ap()
    am_d = nc.dram_tensor("am", [128, 5], f32, kind="ExternalOutput").ap()

    with TileContext(nc) as tc:
        with (
            tc.tile_pool(name="sb", bufs=1) as sbp,
            tc.tile_pool(name="ps", bufs=1, space="PSUM") as psp,
        ):
            am_sb = sbp.tile([128, 5], f32, tag="am")

            lnin_sb, xt_sb, comb_sb, logs_sb, cop_sb, ps_sb = (
                [], [], [], [], [], [])
            for b in range(NB):
                lnin_sb.append(sbp.tile([128, 1920], fp8, tag=f"lnin{b}",
                                        name=f"lnin{b}"))
                xt_sb.append(sbp.tile([128, 2048], fp8, tag=f"xt{b}",
                                      name=f"xt{b}"))
                comb_sb.append(sbp.tile([128, 3072], fp8, tag=f"comb{b}",
                                        name=f"comb{b}"))
                logs_sb.append(sbp.tile([128, 1920], bf16, tag=f"logs{b}",
                                        name=f"logs{b}"))
                cop_sb.append(sbp.tile([128, 768], f16, tag=f"cop{b}",
                                       name=f"cop{b}"))
                ps_sb.append(psp.tile([128, 1024], f32, tag=f"ps{b}",
                                      name=f"ps{b}"))

            # ---- phase A: all input DMAs on the sync queue, in exact
            # consumption order (one queue = strict FIFO = no cross-queue
            # interleave on the shared DMA-engine pipe).
            def dma_ln(b, half):
                lo = b * 1920 + half * 960
                nc.sync.dma_start(lnin_sb[b][:, half * 960:(half + 1) * 960],
                                  lnin_d[:, lo:lo + 960])

            def dma_comb(b):
                nc.sync.dma_start(comb_sb[b][:],
                                  xoc_d[:, b * 3072:(b + 1) * 3072])

            def dma_xt(b):
                nc.sync.dma_start(xt_sb[b][:],
                                  xt_d[:, b * 2048:(b + 1) * 2048])

            dma_ln(0, 0)
            dma_ln(0, 1)
            dma_ln(1, 0)
            dma_ln(1, 1)
            dma_comb(0)
            dma_xt(0)
            dma_ln(2, 0)
            dma_ln(2, 1)
            dma_comb(1)
            dma_xt(1)
            dma_ln(3, 0)
            dma_ln(3, 1)
            dma_comb(2)
            dma_xt(2)
            dma_comb(3)
            dma_xt(3)

            comb_vs = [comb_sb[b][:].rearrange("p (k q) -> p k q", q=384)
                       for b in range(NB)]

            # ---- phase B: per-batch compute
            def mms(b, dlo, dhi):
                xt_v = xt_sb[b][:].rearrange("p (k f) -> p k f", f=256)
                ps = ps_sb[b]
                for d in range(dlo, dhi):
                    st = dict(start=(d == 0), stop=(d == ND - 1))
                    rhs = comb_vs[b][:, 2 * d:2 * d + 2, :]
                    nc.tensor.matmul(ps[:, 0:384],
                                     xt_v[:, 2 * d:2 * d + 2, 0:128], rhs,
                                     perf_mode=DR, **st)
                    nc.tensor.matmul(ps[:, 512:896],
                                     xt_v[:, 2 * d:2 * d + 2, 128:256], rhs,
                                     perf_mode=DR, **st)

            def ps_view(b):
                return ps_sb[b][:].rearrange(
                    "p (h q) -> p h q", q=512)[:, :, 0:384]

            def sub(b, tlo, thi, eng):
                logs = logs_sb[b]
                eng.tensor_sub(comb_vs[b][:, tlo:thi, F:384],
                               logs[:, tlo * F:thi * F],
                               logs[:, 960 + tlo * F:960 + thi * F])

            for b in range(NB):
                logs = logs_sb[b]
                lnin = lnin_sb[b]
                nc.scalar.activation(logs[:, 0:960], lnin[:, 0:960], AF.Ln)
                if b < NB - 1:
                    nc.scalar.activation(
                        logs[:, 960:1920], lnin[:, 960:1920], AF.Ln,
                        accum_out=am_sb[:, b:b + 1])
                    sub(b, 0, 5, nc.vector)
                    # previous batch's PSUM copy issues AFTER this batch's
                    # sub so the in-order DVE stream never delays a sub
                    if b > 0:
                        nc.vector.tensor_copy(cop_sb[b - 1][:], ps_view(b - 1))
                    mms(b, 0, ND)
                else:
                    # split the last batch 4+1 gathered tiles so the serial
                    # tail after the ACT chain is one tiny sub + 4 matmuls
                    nc.scalar.activation(
                        logs[:, 960:1728], lnin[:, 960:1728], AF.Ln,
                        accum_out=am_sb[:, 3:4])
                    sub(b, 0, 4, nc.vector)
                    mms(b, 0, 2)
                    nc.scalar.activation(
                        logs[:, 1728:1920], lnin[:, 1728:1920], AF.Ln,
                        accum_out=am_sb[:, 4:5])
                    sub(b, 4, 5, nc.vector)
                    mms(b, 2, ND)
            # batches 2 and 3's copies on the ACT engine, which is idle once
            # its Ln chain ends; keeps the DVE tail free for batch-3 subs
            nc.scalar.copy(cop_sb[NB - 2][:], ps_view(NB - 2))
            nc.scalar.copy(cop_sb[NB - 1][:], ps_view(NB - 1))

            # ---- phase C: output DMAs
            nc.sync.dma_start(am_d, am_sb[:])
            for b in range(NB):
                nc.sync.dma_start(cop_d[:, b * 768:(b + 1) * 768],
                                  cop_sb[b][:])

    nc.compile()
    return nc


def _get_program():
    global _PROG
    if _PROG is None:
        _PROG = _build_program()
    return _PROG


def kernel(outputs, targets, attention_mask):
    global LAST
    out_np = np.asarray(outputs, dtype=np.float32).reshape(B, S, F)
    tgt_np = np.asarray(targets, dtype=np.float32).reshape(B, S, F)
    m_np = np.asarray(attention_mask)

    # permute rows per batch so unmasked rows come first: cost/G/Amask are
    # all permutation-invariant over s, and the device then only needs Ln
    # over the first 5 s-tiles (640 rows >= max nnz with huge margin)
    order = np.argsort(m_np == 0, axis=1, kind="stable")
    assert int(m_np.sum(axis=1).max()) <= 5 * 128, "gather capacity exceeded"
    out_np = np.take_along_axis(out_np, order[:, :, None], axis=1)
    tgt_np = np.take_along_axis(tgt_np, order[:, :, None], axis=1)
    m_p = np.take_along_axis(m_np, order, axis=1)
    mf = m_p.astype(np.float32)[:, :, None]
    f8 = ml_dtypes.float8_e4m3fn
    # masked Ln inputs; binaries and masked copies are cheap host prep.
    # lnin = [xoo_b | xzo_b] gathered tiles, in exact ACT consumption order.
    ng = 5 * 128
    xoo_g = (out_np[:, 0:ng] * mf[:, 0:ng] + (1.0 - mf[:, 0:ng])).astype(f8)
    xzo_g = ((1.0 - out_np[:, 0:ng]) * mf[:, 0:ng]
             + (1.0 - mf[:, 0:ng])).astype(f8)
    lnin_all = np.concatenate(
        [xoo_g.reshape(B, 1, 5, 128, F),
         xzo_g.reshape(B, 1, 5, 128, F)], axis=1)  # [B, 2, 5, 128, F]
    # xt tiles zero-padded to 256 cols: [hi f0:128 | lo f128:192 | 64 zeros]
    xt_all = np.zeros((B, NT, 128, 256), dtype=f8)
    xt_all[:, :, :, 0:F] = tgt_np.astype(f8).reshape(B, NT, 128, F)
    # comb image: xo tiles in cols 0:192 of each 384 block, zeros in mD slots
    xoc_all = np.zeros((B, NT, 128, 384), dtype=f8)
    xoc_all[:, :, :, 0:F] = out_np.astype(f8).reshape(B, NT, 128, F)

    in_maps = []
    for c in range(NCORE):
        bs = slice(c * NB, (c + 1) * NB)
        in_maps.append({
            "lnin": np.ascontiguousarray(
                lnin_all[bs].transpose(3, 0, 1, 2, 4).reshape(128, NB * 1920)),
            "xt": np.ascontiguousarray(
                xt_all[bs].transpose(2, 0, 1, 3).reshape(128, NB * 2048)),
            "xoc": np.ascontiguousarray(
                xoc_all[bs].transpose(2, 0, 1, 3).reshape(128, NB * 3072)),
        })

    nc = _get_program()
    res = run_bass_kernel_spmd(nc, in_maps, list(range(NCORE)))
    LAST = res

    P = np.array(list(permutations(range(E))), dtype=np.int32)
    t_idx = np.arange(E)[None, :]
    ar = np.arange(E)
    p_arange = np.arange(128)
    diag = p_arange[:, None] % CI + np.arange(E)[None, :] * CI  # [128, 6]
    num = 0.0
    for c in range(NCORE):
        cop = res.results[c]["cop"].astype(np.float32)  # [128, NB*768]
        am = res.results[c]["am"]                       # [128, 5] f32
        for b in range(NB):
            pane = cop[:, b * 768:(b + 1) * 768]
            # pane cols: [cost-hi 0:192 | G-hi 192:384 | cost-lo | G-lo]
            # wanted entry per partition: col g*32 + p%32 of each block
            ch = pane[p_arange[:, None], diag]               # [128, 6]
            gh = pane[p_arange[:, None], 192 + diag]
            cl = pane[p_arange[:64, None], 384 + diag[:64]]  # [64, 6]
            gl = pane[p_arange[:64, None], 576 + diag[:64]]
            cost = -np.concatenate(
                [ch.reshape(4, CI, 6).sum(1, dtype=np.float32),
                 cl.reshape(2, CI, 6).sum(1, dtype=np.float32)], axis=0)
            G = np.concatenate(
                [gh.reshape(4, CI, 6).sum(1, dtype=np.float32),
                 gl.reshape(2, CI, 6).sum(1, dtype=np.float32)], axis=0)

            totals = cost[t_idx, P].sum(-1, dtype=np.float32)
            perm = P[int(np.argmin(totals))]
            num += -0.5 * float(G[ar, perm].sum(dtype=np.float64))
        num += 0.5 * -am.sum(dtype=np.float64)

    den = float(m_np.sum())
    return np.float32(num / den)


# revision 12
# speedup vs baseline: 1.0687x; 1.0687x over previous
"""BiMatchLoss kernel for Trainium2 (8 NeuronCores, SPMD data-parallel over batch).

Math (validated vs reference):
  BCE(p,t) = -log1mp(p) - t*(logp(p) - log1mp(p))
  Summed over a bijective matching perm, the -log1mp part is perm-independent.
  Per batch b the device computes (one pass over the data):
    cost[t,o]  = -sum_{s,ci} tgt[s,t,ci] * out[s,o,ci]            (argmin input)
    G[t,o]     =  sum_{s,ci} tgt[s,t,ci] * mD[s,o,ci]
    Amask      =  sum_{s,o,ci} m[s] * (-log1mp[s,o,ci])
  where mD = m*(logp - log1mp). Host pre-masks the Ln inputs so the device
  computes m*logp = Ln(m*p + 1-m) and m*log1mp = Ln(m*(1-p) + 1-m) directly
  (the (1-p) form keeps fp8 inputs accurate where p -> 1).
  final = sum_b 0.5*(Amask_b - sum_t G[t, perm_b[t]]) / sum(m)

Device per batch: 2 Ln activations over [128,1536] fp8 inputs (ACT; the
log1mp one accumulates Amask row-sums), a subtract writing fp8 mD into the
comb rhs slots, 8 fp8 DoubleRow matmuls (K=256 = two s-tiles per matmul,
PSUM-accumulated over 4 double-tiles; the t4,t5 weight chunk is zero-padded to
M=128 so every PSUM row is written), then ONE PSUM->SBUF f16 copy. The
block-diagonal extraction needs no on-device reduce at all: the wanted
entry per partition is the single element PS[p, g*32 + p%32], so the host
just fancy-indexes the copied [128,768] panes and sums 32 rows. All input
DMAs ride the sync queue as contiguous per-partition lines in exact
consumption order so the ACT chain never stalls. Batch 3's log1mp/sub/
matmuls are split 6+2 tiles and its PSUM copy runs on the then-idle ACT
engine to shorten the serial tail. Host does the 720-permutation argmin and
final scalar assembly.
"""

import os
from itertools import permutations

import numpy as np
import ml_dtypes

import concourse.bacc as bacc
import concourse.mybir as mybir
from concourse.tile import TileContext
from concourse.bass_utils import run_bass_kernel_spmd

B, S, E, C = 32, 1024, 6, 16
F = E * C * 2          # 192 flattened (e, c, i)
CI = C * 2             # 32
NCORE = 8
NB = B // NCORE        # 4 batches per core
NT = S // 128          # 8 s-tiles per batch
ND = NT // 2           # 4 double-tiles (K=256) per batch

f32 = mybir.dt.float32
f16 = mybir.dt.float16
bf16 = mybir.dt.bfloat16
fp8 = mybir.dt.float8e4
AF = mybir.ActivationFunctionType
ALU = mybir.AluOpType
DR = mybir.MatmulPerfMode.DoubleRow

_PROG = None           # cached compiled Bass program
LAST = None            # last BassKernelResults (for test.py timing)


def _build_program():
    nc = bacc.Bacc("TRN2", target_bir_lowering=False, debug=False,
                   num_devices=1)

    lnin_d = nc.dram_tensor("lnin", [128, NB * 1920], fp8,
                            kind="ExternalInput").ap()
    xt_d = nc.dram_tensor("xt", [128, NB * 2048], fp8,
                          kind="ExternalInput").ap()
    xoc_d = nc.dram_tensor("xoc", [128, NB * 3072], fp8,
                           kind="ExternalInput").ap()
    cop_d = nc.dram_tensor("cop", [128, NB * 768], f16,
                           kind="ExternalOutput").ap()
    am_d = nc.dram_tensor("am", [128, 5], f32, kind="ExternalOutput").ap()

    with TileContext(nc) as tc:
        with (
            tc.tile_pool(name="sb", bufs=1) as sbp,
            tc.tile_pool(name="ps", bufs=1, space="PSUM") as psp,
        ):
            am_sb = sbp.tile([128, 5], f32, tag="am")

            lnin_sb, xt_sb, comb_sb, logs_sb, cop_sb, ps_sb = (
                [], [], [], [], [], [])
            for b in range(NB):
                lnin_sb.append(sbp.tile([128, 1920], fp8, tag=f"lnin{b}",
                                        name=f"lnin{b}"))
                xt_sb.append(sbp.tile([128, 2048], fp8, tag=f"xt{b}",
                                      name=f"xt{b}"))
                comb_sb.append(sbp.tile([128, 3072], fp8, tag=f"comb{b}",
                                        name=f"comb{b}"))
                logs_sb.append(sbp.tile([128, 1920], bf16, tag=f"logs{b}",
                                        name=f"logs{b}"))
                cop_sb.append(sbp.tile([128, 768], f16, tag=f"cop{b}",
                                       name=f"cop{b}"))
                ps_sb.append(psp.tile([128, 1024], f32, tag=f"ps{b}",
                                      name=f"ps{b}"))

            # ---- phase A: all input DMAs on the sync queue, in exact
            # consumption order (one queue = strict FIFO = no cross-queue
            # interleave on the shared DMA-engine pipe).
            def dma_ln(b, half):
                lo = b * 1920 + half * 960
                nc.sync.dma_start(lnin_sb[b][:, half * 960:(half + 1) * 960],
                                  lnin_d[:, lo:lo + 960])

            def dma_comb(b):
                nc.sync.dma_start(comb_sb[b][:],
                                  xoc_d[:, b * 3072:(b + 1) * 3072])

            def dma_xt(b):
                nc.sync.dma_start(xt_sb[b][:],
                                  xt_d[:, b * 2048:(b + 1) * 2048])

            dma_ln(0, 0)
            dma_ln(0, 1)
            dma_ln(1, 0)
            dma_ln(1, 1)
            dma_comb(0)
            dma_xt(0)
            dma_ln(2, 0)
            dma_ln(2, 1)
            dma_comb(1)
            dma_xt(1)
            dma_ln(3, 0)
            dma_ln(3, 1)
            dma_comb(2)
            dma_xt(2)
            dma_comb(3)
            dma_xt(3)

            comb_vs = [comb_sb[b][:].rearrange("p (k q) -> p k q", q=384)
                       for b in range(NB)]

            # ---- phase B: per-batch compute
            def mms(b, dlo, dhi):
                xt_v = xt_sb[b][:].rearrange("p (k f) -> p k f", f=256)
                ps = ps_sb[b]
                for d in range(dlo, dhi):
                    st = dict(start=(d == 0), stop=(d == ND - 1))
                    rhs = comb_vs[b][:, 2 * d:2 * d + 2, :]
                    nc.tensor.matmul(ps[:, 0:384],
                                     xt_v[:, 2 * d:2 * d + 2, 0:128], rhs,
                                     perf_mode=DR, **st)
                    nc.tensor.matmul(ps[:, 512:896],
                                     xt_v[:, 2 * d:2 * d + 2, 128:256], rhs,
                                     perf_mode=DR, **st)

            def ps_view(b):
                return ps_sb[b][:].rearrange(
                    "p (h q) -> p h q", q=512)[:, :, 0:384]

            def sub(b, tlo, thi, eng):
                logs = logs_sb[b]
                with tc.high_priority():
                    eng.tensor_sub(comb_vs[b][:, tlo:thi, F:384],
                                   logs[:, tlo * F:thi * F],
                                   logs[:, 960 + tlo * F:960 + thi * F])

            for b in range(NB):
                logs = logs_sb[b]
                lnin = lnin_sb[b]
                nc.scalar.activation(logs[:, 0:960], lnin[:, 0:960], AF.Ln)
                if b < NB - 1:
                    nc.scalar.activation(
                        logs[:, 960:1920], lnin[:, 960:1920], AF.Ln,
                        accum_out=am_sb[:, b:b + 1])
                    sub(b, 0, 5, nc.vector)
                    mms(b, 0, ND)
                else:
                    # split the last batch 4+1 gathered tiles so the serial
                    # tail after the ACT chain is one tiny sub + 4 matmuls
                    nc.scalar.activation(
                        logs[:, 960:1728], lnin[:, 960:1728], AF.Ln,
                        accum_out=am_sb[:, 3:4])
                    sub(b, 0, 4, nc.vector)
                    mms(b, 0, 2)
                    nc.scalar.activation(
                        logs[:, 1728:1920], lnin[:, 1728:1920], AF.Ln,
                        accum_out=am_sb[:, 4:5])
                    sub(b, 4, 5, nc.vector)
                    mms(b, 2, ND)
            # all PSUM copies on the ACT engine, which is idle once its Ln
            # chain ends; the DVE stream then carries only the subs, so the
            # scheduler cannot delay a sub behind a copy
            for b in range(NB):
                nc.scalar.copy(cop_sb[b][:], ps_view(b))

            # ---- phase C: output DMAs
            nc.sync.dma_start(am_d, am_sb[:])
            for b in range(NB):
                nc.sync.dma_start(cop_d[:, b * 768:(b + 1) * 768],
                                  cop_sb[b][:])

    nc.compile()
    return nc


def _get_program():
    global _PROG
    if _PROG is None:
        _PROG = _build_program()
    return _PROG


def kernel(outputs, targets, attention_mask):
    global LAST
    out_np = np.asarray(outputs, dtype=np.float32).reshape(B, S, F)
    tgt_np = np.asarray(targets, dtype=np.float32).reshape(B, S, F)
    m_np = np.asarray(attention_mask)

    # permute rows per batch so unmasked rows come first: cost/G/Amask are
    # all permutation-invariant over s, and the device then only needs Ln
    # over the first 5 s-tiles (640 rows >= max nnz with huge margin)
    order = np.argsort(m_np == 0, axis=1, kind="stable")
    assert int(m_np.sum(axis=1).max()) <= 5 * 128, "gather capacity exceeded"
    out_np = np.take_along_axis(out_np, order[:, :, None], axis=1)
    tgt_np = np.take_along_axis(tgt_np, order[:, :, None], axis=1)
    m_p = np.take_along_axis(m_np, order, axis=1)
    mf = m_p.astype(np.float32)[:, :, None]
    f8 = ml_dtypes.float8_e4m3fn
    # masked Ln inputs; binaries and masked copies are cheap host prep.
    # lnin = [xoo_b | xzo_b] gathered tiles, in exact ACT consumption order.
    ng = 5 * 128
    xoo_g = (out_np[:, 0:ng] * mf[:, 0:ng] + (1.0 - mf[:, 0:ng])).astype(f8)
    xzo_g = ((1.0 - out_np[:, 0:ng]) * mf[:, 0:ng]
             + (1.0 - mf[:, 0:ng])).astype(f8)
    lnin_all = np.concatenate(
        [xoo_g.reshape(B, 1, 5, 128, F),
         xzo_g.reshape(B, 1, 5, 128, F)], axis=1)  # [B, 2, 5, 128, F]
    # xt tiles zero-padded to 256 cols: [hi f0:128 | lo f128:192 | 64 zeros]
    xt_all = np.zeros((B, NT, 128, 256), dtype=f8)
    xt_all[:, :, :, 0:F] = tgt_np.astype(f8).reshape(B, NT, 128, F)
    # comb image: xo tiles in cols 0:192 of each 384 block, zeros in mD slots
    xoc_all = np.zeros((B, NT, 128, 384), dtype=f8)
    xoc_all[:, :, :, 0:F] = out_np.astype(f8).reshape(B, NT, 128, F)

    in_maps = []
    for c in range(NCORE):
        bs = slice(c * NB, (c + 1) * NB)
        in_maps.append({
            "lnin": np.ascontiguousarray(
                lnin_all[bs].transpose(3, 0, 1, 2, 4).reshape(128, NB * 1920)),
            "xt": np.ascontiguousarray(
                xt_all[bs].transpose(2, 0, 1, 3).reshape(128, NB * 2048)),
            "xoc": np.ascontiguousarray(
                xoc_all[bs].transpose(2, 0, 1, 3).reshape(128, NB * 3072)),
        })

    nc = _get_program()
    res = run_bass_kernel_spmd(nc, in_maps, list(range(NCORE)))
    LAST = res

    P = np.array(list(permutations(range(E))), dtype=np.int32)
    t_idx = np.arange(E)[None, :]
    ar = np.arange(E)
    p_arange = np.arange(128)
    diag = p_arange[:, None] % CI + np.arange(E)[None, :] * CI  # [128, 6]
    num = 0.0
    for c in range(NCORE):
        cop = res.results[c]["cop"].astype(np.float32)  # [128, NB*768]
        am = res.results[c]["am"]                       # [128, 5] f32
        for b in range(NB):
            pane = cop[:, b * 768:(b + 1) * 768]
            # pane cols: [cost-hi 0:192 | G-hi 192:384 | cost-lo | G-lo]
            # wanted entry per partition: col g*32 + p%32 of each block
            ch = pane[p_arange[:, None], diag]               # [128, 6]
            gh = pane[p_arange[:, None], 192 + diag]
            cl = pane[p_arange[:64, None], 384 + diag[:64]]  # [64, 6]
            gl = pane[p_arange[:64, None], 576 + diag[:64]]
            cost = -np.concatenate(
                [ch.reshape(4, CI, 6).sum(1, dtype=np.float32),
                 cl.reshape(2, CI, 6).sum(1, dtype=np.float32)], axis=0)
            G = np.concatenate(
                [gh.reshape(4, CI, 6).sum(1, dtype=np.float32),
                 gl.reshape(2, CI, 6).sum(1, dtype=np.float32)], axis=0)

            totals = cost[t_idx, P].sum(-1, dtype=np.float32)
            perm = P[int(np.argmin(totals))]
            num += -0.5 * float(G[ar, perm].sum(dtype=np.float64))
        num += 0.5 * -am.sum(dtype=np.float64)

    den = float(m_np.sum())
    return np.float32(num / den)
